# revision 7
# baseline (speedup 1.0000x reference)
"""3D Gaussian blur (kernel_size=5, sigma=1.0) on (2,1,192,256,256) f32,
distributed over 8 Trainium2 NeuronCores.

The torch kernel factors: g[i,j,l] = aD[i] * (1/5) * bW[l] -> separable into
Gaussian along D, box along H, Gaussian along W.

Per-core (2 batches x 4 D-slabs of 48):

Phase A' fuses the H box conv AND the D gauss conv into the matmul
contraction dim: stationary tiles pack (8 d-slices x 16 h-rows) into the
128 partitions, the moving operand is a constant band over
(12 output-slice slots x 20 h-out cols), PSUM accumulates across d-blocks
and h-tiles via per-element has_written semantics (start=True only on the
first matmul touching each bank per epoch).  Output q[w, o, ho] lands
w-major in PSUM, is evacuated to SBUF bf16 in 8 h-regions of 36 cols with
4-col overlaps resolved by gpsimd fixup ops.

Phase B does the W gauss conv: stationary q-tiles [w,128 h-chunk], moving
band [w,130], PSUM accumulates 4 MMs per output slice; evac to bf16 and
DMA out h-major.
"""
import numpy as np
import ml_dtypes

import concourse.bacc as bacc
import concourse.tile as tile
from concourse import mybir
from concourse.bass_utils import run_bass_kernel_spmd

B = 2          # batch
D = 192        # depth
HW = 256       # height = width
SLAB = 48      # output slices per core
SPAD = 56      # input slices incl. 2+2 conv halo and 4 block pad
NBK = 7        # d-blocks of 8
NE = 8         # h-eighths (regions of 32 h + 4 halo = 36 cols)
P = 128
N_CORES = 8
NB = 130       # pass-B band cols

F32 = mybir.dt.float32
BF16 = mybir.dt.bfloat16

# pass A' piece table: bk -> [(o_lo, o_hi, bank, start_flag)]
PIECES = {
    0: [(0, 7, 0, True)],
    1: [(4, 13, 0, False), (14, 15, 1, True)],
    2: [(12, 13, 0, False), (14, 23, 1, False)],
    3: [(20, 27, 1, False), (28, 31, 2, True)],
    4: [(28, 39, 2, False)],
    5: [(36, 41, 2, False), (42, 47, 3, True)],
    6: [(44, 47, 3, False)],
}
# bank -> (last bk writing it, first slot, n slots)
EVAC = {0: (2, 0, 14), 1: (3, 14, 14), 2: (5, 28, 14), 3: (6, 42, 6)}


def _taps():
    c = np.arange(5, dtype=np.float64) - 2
    u = np.exp(-c * c / 2.0)   # D-axis Gaussian (sigma=1)
    v = np.exp(-c * c)         # W-axis Gaussian (sigma^2=1/2)
    aD = (u / u.sum()).astype(np.float64)
    bW = (v / v.sum()).astype(np.float64)
    return aD, bW


def _const_tensors():
    aD, bW = _taps()
    # A' band [128=(sl,hl), 12 sigma, 20 r]: aD[sl+4-sg] * 0.2 * [0<=hl+4-r<=4]
    ba = np.zeros((P, 12, 20), dtype=np.float64)
    for sl in range(8):
        for hl in range(16):
            p = sl * 16 + hl
            for sg in range(12):
                kd = sl + 4 - sg
                if not (0 <= kd <= 4):
                    continue
                for r in range(20):
                    kh = hl + 4 - r
                    if 0 <= kh <= 4:
                        ba[p, sg, r] = aD[kd] * 0.2
    # B bands [2 wb, 128, 130]
    bw = np.zeros((2, P, NB), dtype=np.float64)
    for w in range(P):
        for c in range(NB):
            k0 = w - c + 2        # wb0: wo = c
            if 0 <= k0 <= 4:
                bw[0, w, c] = bW[k0]
            k1 = w - c + 4        # wb1: w = 128+wl, wo = 126+c
            if 0 <= k1 <= 4:
                bw[1, w, c] = bW[k1]
    return (ba.astype(ml_dtypes.bfloat16), bw.astype(ml_dtypes.bfloat16))


def _build_nc():
    nc = bacc.Bacc("TRN2", target_bir_lowering=False, debug=False,
                   num_devices=N_CORES)
    # x[p=(s%8)*16+h%16, wb, e, bk, tl, wc]
    x_d = nc.declare_dram_parameter("x", [P, 2, NE, NBK, 2, P], BF16,
                                    isOutput=False)
    ba_d = nc.declare_dram_parameter("ba", [P, 12, 20], BF16, isOutput=False)
    bw_d = nc.declare_dram_parameter("bw", [2, P, NB], BF16, isOutput=False)
    # out[p=h%128, o, hb=h//128, w]  (bf16)
    out_d = nc.declare_dram_parameter("out", [P, SLAB, 2, HW], BF16,
                                      isOutput=True)

    with tile.TileContext(nc) as tc:
        with (
            tc.tile_pool(name="consts", bufs=1) as cpool,
            tc.tile_pool(name="xcols", bufs=1) as xpool,
            tc.tile_pool(name="q", bufs=1) as qpool,
            tc.tile_pool(name="ost", bufs=2) as opool,
            tc.tile_pool(name="pa", bufs=1, space="PSUM") as papool,
            tc.tile_pool(name="pb", bufs=3, space="PSUM") as pbpool,
        ):
            ba_sb = cpool.tile([P, 12, 20], BF16, tag="ba")
            bw_sb = cpool.tile([P, 2, NB], BF16, tag="bw")
            nc.sync.dma_start(ba_sb[:], ba_d[:])
            nc.sync.dma_start(bw_sb[:, 0], bw_d[0])
            nc.sync.dma_start(bw_sb[:, 1], bw_d[1])

            # q[wp, wb, o, e, 36]
            q_sb = qpool.tile([P, 2, SLAB, NE, 36], BF16, tag="q")

            # all 16 column DMAs dispatched up front
            xcols = {}
            for wb in range(2):
                for e in range(NE):
                    xc = xpool.tile([P, NBK, 2, P], BF16, tag=f"x{wb}{e}")
                    xcols[(wb, e)] = xc
                    nc.sync.dma_start(xc[:], x_d[:, wb, e])

            paA = papool.tile([P, 4, 512], F32, tag="pa")  # 4 banks

            # ---------------- phase A' ----------------
            nev = 0
            for wb in range(2):
                for e in range(NE):
                    xc = xcols[(wb, e)]
                    for bk in range(NBK):
                        for tl in range(2):
                            lhsT = xc[:, bk, tl, :]
                            r0 = 2 if (e == 0 and tl == 0) else 0
                            r1 = 18 if (e == 7 and tl == 1) else 20
                            for (olo, ohi, bank, st) in PIECES[bk]:
                                ns = ohi - olo + 1
                                sg0 = olo - (8 * bk - 4)
                                s0 = olo % 14
                                sub = paA[:, bank, s0 * 36:(s0 + ns) * 36]
                                sub = sub.rearrange("p (s c) -> p s c",
                                                    s=ns, c=36)
                                nc.tensor.matmul(
                                    sub[:, :, tl * 16 + r0: tl * 16 + r1],
                                    lhsT,
                                    ba_sb[:, sg0:sg0 + ns, r0:r1],
                                    start=(st and tl == 0), stop=True,
                                    skip_group_check=True)
                    # emit evacs for this column
                    for bnk, (lastbk, slot0, nsl) in EVAC.items():
                        src = paA[:, bnk, 0:nsl * 36].rearrange(
                            "p (s c) -> p s c", s=nsl, c=36)
                        dst = q_sb[:, wb, slot0:slot0 + nsl, e, :]
                        if nev % 2 == 0:
                            nc.vector.tensor_copy(dst, src)
                        else:
                            nc.scalar.copy(dst, src)
                        nev += 1
                    # overlap fixups for boundary (e-1, e)
                    if e >= 1:
                        left = q_sb[:, wb, :, e - 1, 32:36]
                        right = q_sb[:, wb, :, e, 0:4]
                        nc.vector.scalar_tensor_tensor(
                            right, left, 1.0, right,
                            mybir.AluOpType.mult, mybir.AluOpType.add)
                        nc.gpsimd.tensor_copy(left, right)

            # ---------------- phase B ----------------
            ost = None
            for o in range(SLAB):
                ps = pbpool.tile([P, 2, HW], F32, tag="pb")
                nmm = 0
                for hb in range(2):
                    for wb in range(2):
                        c0 = 0 if wb == 0 else HW - NB
                        for j in range(4):
                            # stationary must be 1 free dim: col-tile M=32
                            lhsT = q_sb[:, wb, o, 4 * hb + j, 2:34]
                            # col-tiled start=True clears has_written only for
                            # its own 32 partitions: each j needs its own start
                            nc.tensor.matmul(
                                ps[32 * j:32 * j + 32, hb, c0:c0 + NB],
                                lhsT,
                                bw_sb[:, wb, :],
                                start=(hb == 0 and wb == 0),
                                stop=(nmm == 15),
                                tile_position=(0, 32 * j),
                                skip_group_check=True)
                            nmm += 1
                if o % 8 == 0:
                    ost = opool.tile([P, 8, 2, HW], BF16, tag="ost")
                if o % 2 == 0:
                    nc.vector.tensor_copy(ost[:, o % 8], ps[:])
                else:
                    nc.scalar.copy(ost[:, o % 8], ps[:])
                if o % 8 == 7:
                    nc.gpsimd.dma_start(out_d[:, o - 7:o + 1], ost[:])

    nc.compile()
    return nc


_NC_CACHE = {}


def _get_nc():
    if "nc" not in _NC_CACHE:
        _NC_CACHE["nc"] = _build_nc()
    return _NC_CACHE["nc"]


def kernel(x, kernel_size, _trace=False, _trace_kwargs=None):
    """x: (2, 1, 192, 256, 256) float32; kernel_size: 5. Returns same shape."""
    assert int(kernel_size) == 5, "kernel hardcodes kernel_size=5"
    x = np.asarray(x)
    assert x.shape == (B, 1, D, HW, HW), x.shape
    in_dtype = x.dtype

    nc = _get_nc()
    ba, bw = _const_tensors()

    xb = np.asarray(x[:, 0]).astype(ml_dtypes.bfloat16)

    in_maps = []
    for c in range(N_CORES):
        b, j = divmod(c, 4)
        lo = SLAB * j - 2
        xp = np.zeros((SPAD, HW, HW), dtype=ml_dtypes.bfloat16)
        g0, g1 = max(0, lo), min(D, lo + SPAD)
        xp[g0 - lo:g1 - lo] = xb[b, g0:g1]
        # [bk, sl, e, tl, hl, wb, wc] -> [sl, hl, wb, e, bk, tl, wc]
        sw = xp.reshape(NBK, 8, NE, 2, 16, 2, P).transpose(1, 4, 5, 2, 0, 3, 6)
        sw = np.ascontiguousarray(sw).reshape(P, 2, NE, NBK, 2, P)
        in_maps.append({"x": sw, "ba": ba, "bw": bw})

    res = run_bass_kernel_spmd(
        nc, in_maps, core_ids=list(range(N_CORES)),
        trace=_trace, **(_trace_kwargs or {}))

    out = np.empty((B, 1, D, HW, HW), dtype=np.float32)
    for c in range(N_CORES):
        b, j = divmod(c, 4)
        r = np.asarray(res.results[c]["out"]).astype(np.float32)
        out[b, 0, j * SLAB:(j + 1) * SLAB] = (
            r.transpose(1, 2, 0, 3).reshape(SLAB, HW, HW))

    if _trace:
        kernel._last_result = res
    return out.astype(in_dtype, copy=False)


# revision 10
# speedup vs baseline: 1.3374x; 1.3374x over previous
"""3D Gaussian blur (kernel_size=5, sigma=1.0) on (2,1,192,256,256) f32,
distributed over 8 Trainium2 NeuronCores.

The torch kernel factors: g[i,j,l] = aD[i] * (1/5) * bW[l] -> separable into
Gaussian along D, box along H, Gaussian along W.

Per-core (2 batches x 4 D-slabs of 48):

Phase A' fuses the H box conv AND the D gauss conv into the matmul
contraction dim: stationary tiles pack (8 d-slices x 16 h-rows) into the
128 partitions, the moving operand is a constant band over
(12 output-slice slots x 20 h-out cols), PSUM accumulates across d-blocks
and h-tiles via per-element has_written semantics (start=True only on the
first matmul touching each bank per epoch).  Output q[w, o, ho] lands
w-major in PSUM, is evacuated to SBUF bf16 in 8 h-regions of 36 cols with
4-col overlaps resolved by gpsimd fixup ops.

Phase B does the W gauss conv: stationary q-tiles [w,128 h-chunk], moving
band [w,130], PSUM accumulates 4 MMs per output slice; evac to bf16 and
DMA out h-major.
"""
import numpy as np
import ml_dtypes

import concourse.bacc as bacc
import concourse.tile as tile
from concourse import mybir
from concourse.bass_utils import run_bass_kernel_spmd

B = 2          # batch
D = 192        # depth
HW = 256       # height = width
SLAB = 48      # output slices per core
SPAD = 56      # input slices incl. 2+2 conv halo and 4 block pad
NBK = 7        # d-blocks of 8
NE = 8         # h-eighths (regions of 32 h + 4 halo = 36 cols)
P = 128
N_CORES = 8
NB = 130       # pass-B band cols

F32 = mybir.dt.float32
BF16 = mybir.dt.bfloat16

# pass A' piece table: bk -> [(o_lo, o_hi, bank, start_flag)]
PIECES = {
    0: [(0, 7, 0, True)],
    1: [(4, 13, 0, False), (14, 15, 1, True)],
    2: [(12, 13, 0, False), (14, 23, 1, False)],
    3: [(20, 27, 1, False), (28, 31, 2, True)],
    4: [(28, 39, 2, False)],
    5: [(36, 41, 2, False), (42, 47, 3, True)],
    6: [(44, 47, 3, False)],
}
# bank -> (last bk writing it, first slot, n slots)
EVAC = {0: (2, 0, 14), 1: (3, 14, 14), 2: (5, 28, 14), 3: (6, 42, 6)}


def _taps():
    c = np.arange(5, dtype=np.float64) - 2
    u = np.exp(-c * c / 2.0)   # D-axis Gaussian (sigma=1)
    v = np.exp(-c * c)         # W-axis Gaussian (sigma^2=1/2)
    aD = (u / u.sum()).astype(np.float64)
    bW = (v / v.sum()).astype(np.float64)
    return aD, bW


def _const_tensors():
    aD, bW = _taps()
    # A' band [128=(sl,hl), 12 sigma, 20 r]: aD[sl+4-sg] * 0.2 * [0<=hl+4-r<=4]
    ba = np.zeros((P, 12, 20), dtype=np.float64)
    for sl in range(8):
        for hl in range(16):
            p = sl * 16 + hl
            for sg in range(12):
                kd = sl + 4 - sg
                if not (0 <= kd <= 4):
                    continue
                for r in range(20):
                    kh = hl + 4 - r
                    if 0 <= kh <= 4:
                        ba[p, sg, r] = aD[kd] * 0.2
    # B bands [2 wb, 128, 130]
    bw = np.zeros((2, P, NB), dtype=np.float64)
    for w in range(P):
        for c in range(NB):
            k0 = w - c + 2        # wb0: wo = c
            if 0 <= k0 <= 4:
                bw[0, w, c] = bW[k0]
            k1 = w - c + 4        # wb1: w = 128+wl, wo = 126+c
            if 0 <= k1 <= 4:
                bw[1, w, c] = bW[k1]
    return (ba.astype(ml_dtypes.bfloat16), bw.astype(ml_dtypes.bfloat16))


def _build_nc():
    nc = bacc.Bacc("TRN2", target_bir_lowering=False, debug=False,
                   num_devices=N_CORES)
    # x[p=(s%8)*16+h%16, wb, e, bk, tl, wc]
    x_d = nc.declare_dram_parameter("x", [P, 2, NE, NBK, 2, P], BF16,
                                    isOutput=False)
    ba_d = nc.declare_dram_parameter("ba", [P, 12, 20], BF16, isOutput=False)
    bw_d = nc.declare_dram_parameter("bw", [2, P, NB], BF16, isOutput=False)
    # out[p=h%128, o, hb=h//128, w]  (bf16)
    out_d = nc.declare_dram_parameter("out", [P, SLAB, 2, HW], BF16,
                                      isOutput=True)

    with tile.TileContext(nc) as tc:
        with (
            tc.tile_pool(name="consts", bufs=1) as cpool,
            tc.tile_pool(name="xcols", bufs=1) as xpool,
            tc.tile_pool(name="q", bufs=1) as qpool,
            tc.tile_pool(name="ost", bufs=2) as opool,
            tc.tile_pool(name="pa", bufs=3, space="PSUM") as papool,
            tc.tile_pool(name="pb", bufs=2, space="PSUM") as pbpool,
        ):
            ba_sb = cpool.tile([P, 12, 20], BF16, tag="ba")
            bw_sb = cpool.tile([P, 2, NB], BF16, tag="bw")
            nc.sync.dma_start(ba_sb[:], ba_d[:])
            nc.sync.dma_start(bw_sb[:, 0], bw_d[0])
            nc.sync.dma_start(bw_sb[:, 1], bw_d[1])

            # q[wp, wb, o, e, 36]
            q_sb = qpool.tile([P, 2, SLAB, NE, 36], BF16, tag="q")

            # all 16 column DMAs dispatched up front
            xcols = {}
            for wb in range(2):
                for e in range(NE):
                    xc = xpool.tile([P, NBK, 2, P], BF16, tag=f"x{wb}{e}")
                    xcols[(wb, e)] = xc
                    nc.sync.dma_start(xc[:], x_d[:, wb, e])

            # ---------------- phase A' ----------------
            # per column: lo half-tile holds o-slots 0..27 (banks 0..1),
            # hi half-tile holds slots 28..47 (banks 2..3); bufs=3 rotates
            # so next column's MMs overlap this column's evacuations.
            nev = 0
            for wb in range(2):
                for e in range(NE):
                    xc = xcols[(wb, e)]
                    pa_lo = papool.tile([P, 2, 512], F32, tag="pa")
                    pa_hi = papool.tile([P, 2, 512], F32, tag="pa")

                    def _evac(tile_, lb, slot0, nsl):
                        nonlocal nev
                        src = tile_[:, lb, 0:nsl * 36].rearrange(
                            "p (s c) -> p s c", s=nsl, c=36)
                        dst = q_sb[:, wb, slot0:slot0 + nsl, e, :]
                        if nev % 2 == 0:
                            nc.vector.tensor_copy(dst, src)
                        else:
                            nc.scalar.copy(dst, src)
                        nev += 1

                    for bk in range(NBK):
                        for tl in range(2):
                            lhsT = xc[:, bk, tl, :]
                            r0 = 2 if (e == 0 and tl == 0) else 0
                            r1 = 18 if (e == 7 and tl == 1) else 20
                            for (olo, ohi, bank, st) in PIECES[bk]:
                                ns = ohi - olo + 1
                                sg0 = olo - (8 * bk - 4)
                                s0 = olo % 14
                                pt = pa_lo if bank < 2 else pa_hi
                                lb = bank % 2
                                sub = pt[:, lb, s0 * 36:(s0 + ns) * 36]
                                sub = sub.rearrange("p (s c) -> p s c",
                                                    s=ns, c=36)
                                nc.tensor.matmul(
                                    sub[:, :, tl * 16 + r0: tl * 16 + r1],
                                    lhsT,
                                    ba_sb[:, sg0:sg0 + ns, r0:r1],
                                    start=(st and tl == 0), stop=True,
                                    skip_group_check=True)
                        # inline evacs as soon as a bank is complete
                        if bk == 2:
                            _evac(pa_lo, 0, 0, 14)
                        elif bk == 3:
                            _evac(pa_lo, 1, 14, 14)
                        elif bk == 5:
                            _evac(pa_hi, 0, 28, 14)
                        elif bk == 6:
                            _evac(pa_hi, 1, 42, 6)
                    # overlap fixups for boundary (e-1, e)
                    if e >= 1:
                        left = q_sb[:, wb, :, e - 1, 32:36]
                        right = q_sb[:, wb, :, e, 0:4]
                        nc.vector.scalar_tensor_tensor(
                            right, left, 1.0, right,
                            mybir.AluOpType.mult, mybir.AluOpType.add)
                        nc.gpsimd.tensor_copy(left, right)

            # ---------------- phase B ----------------
            ost = None
            for o in range(SLAB):
                ps = pbpool.tile([P, 2, HW], F32, tag="pb")
                nmm = 0
                for hb in range(2):
                    for wb in range(2):
                        c0 = 0 if wb == 0 else HW - NB
                        for j in range(4):
                            # stationary must be 1 free dim: col-tile M=32
                            lhsT = q_sb[:, wb, o, 4 * hb + j, 2:34]
                            # col-tiled start=True clears has_written only for
                            # its own 32 partitions: each j needs its own start
                            nc.tensor.matmul(
                                ps[32 * j:32 * j + 32, hb, c0:c0 + NB],
                                lhsT,
                                bw_sb[:, wb, :],
                                start=(hb == 0 and wb == 0),
                                stop=(nmm == 15),
                                tile_position=(0, 32 * j),
                                skip_group_check=True)
                            nmm += 1
                if o % 8 == 0:
                    ost = opool.tile([P, 8, 2, HW], BF16, tag="ost")
                # split the evac across both engines per slice
                nc.vector.tensor_copy(ost[:, o % 8, 0], ps[:, 0])
                nc.scalar.copy(ost[:, o % 8, 1], ps[:, 1])
                if o % 8 == 7:
                    nc.gpsimd.dma_start(out_d[:, o - 7:o + 1], ost[:])

    nc.compile()
    return nc


_NC_CACHE = {}


def _get_nc():
    if "nc" not in _NC_CACHE:
        _NC_CACHE["nc"] = _build_nc()
    return _NC_CACHE["nc"]


def kernel(x, kernel_size, _trace=False, _trace_kwargs=None):
    """x: (2, 1, 192, 256, 256) float32; kernel_size: 5. Returns same shape."""
    assert int(kernel_size) == 5, "kernel hardcodes kernel_size=5"
    x = np.asarray(x)
    assert x.shape == (B, 1, D, HW, HW), x.shape
    in_dtype = x.dtype

    nc = _get_nc()
    ba, bw = _const_tensors()

    xb = np.asarray(x[:, 0]).astype(ml_dtypes.bfloat16)

    in_maps = []
    for c in range(N_CORES):
        b, j = divmod(c, 4)
        lo = SLAB * j - 2
        xp = np.zeros((SPAD, HW, HW), dtype=ml_dtypes.bfloat16)
        g0, g1 = max(0, lo), min(D, lo + SPAD)
        xp[g0 - lo:g1 - lo] = xb[b, g0:g1]
        # [bk, sl, e, tl, hl, wb, wc] -> [sl, hl, wb, e, bk, tl, wc]
        sw = xp.reshape(NBK, 8, NE, 2, 16, 2, P).transpose(1, 4, 5, 2, 0, 3, 6)
        sw = np.ascontiguousarray(sw).reshape(P, 2, NE, NBK, 2, P)
        in_maps.append({"x": sw, "ba": ba, "bw": bw})

    res = run_bass_kernel_spmd(
        nc, in_maps, core_ids=list(range(N_CORES)),
        trace=_trace, **(_trace_kwargs or {}))

    out = np.empty((B, 1, D, HW, HW), dtype=np.float32)
    for c in range(N_CORES):
        b, j = divmod(c, 4)
        r = np.asarray(res.results[c]["out"]).astype(np.float32)
        out[b, 0, j * SLAB:(j + 1) * SLAB] = (
            r.transpose(1, 2, 0, 3).reshape(SLAB, HW, HW))

    if _trace:
        kernel._last_result = res
    return out.astype(in_dtype, copy=False)
